# revision 79
# baseline (speedup 1.0000x reference)
"""ArcFace loss kernel for 8 Trainium2 NeuronCores.

Strategy (class-parallel, Partial-FC style):
  - weight [100000, 512] is sharded along the class axis: 12500 classes per
    core (padded to 12544 = 7*1792). Shards are passed host-normalized,
    host-transposed ([D, Cpad]) in fp16 so the device streams them straight
    into the TensorEngine as the moving operand.
  - input [512, 512] is normalized and scaled by S on the host, transposed to
    [D, B] fp16, and broadcast to all cores as the stationary operand.
  - Each core computes out[b, c] = <S*in_hat_b, w_hat_c> for its class range
    in natural [B, Cpad] layout: per (super, b-tile) the 4 contraction chunks
    are accumulated into a 4-bank PSUM tile (4 groups of 448 classes), then
    evacuated to fp16 SBUF (VectorE + ScalarE split) and DMA'd out.
  - Stationary = input keeps LDWEIGHTS cheap; dummy warm-up matmuls hold
    the PE p-state (2.4GHz needs 3us of continuous work) while the first
    weight DMAs land on the sync queue, the only fast DMA queue. The final
    tile uses 8 finer PSUM groups, g-outer, so its copies pipeline with the
    matmuls and the output drain starts right after the last matmul.
  - The ArcFace margin only affects one element per row (b, label[b]); the
    host applies the phi transform to those 512 gathered cosines in float64.
"""

import math
import os
import sys

import numpy as np

for _p in ("/opt/trn_rl_repo",):
    if os.path.isdir(_p) and _p not in sys.path:
        sys.path.insert(0, _p)

S = 30.0
MARGIN = 0.5
COS_M = math.cos(MARGIN)
SIN_M = math.sin(MARGIN)
TH = math.cos(math.pi - MARGIN)
MM = math.sin(math.pi - MARGIN) * MARGIN

B, D, C = 512, 512, 100000
NCORES = 8
CSH = C // NCORES            # 12500 classes per core
SUP = 7                      # weight "supers" per core
SUPC = 1792                  # classes per super
CPAD = SUP * SUPC            # 12544
GRP = 4                      # PSUM banks (class groups) per super
GN = SUPC // GRP             # 448 classes per group
BT = B // 128                # 4 batch tiles
DCH = D // 128               # 4 contraction chunks
NWARM = 22                   # PE warm-up matmuls

LAST_RESULT = None
_CACHE = {}


def _build_nc():
    from concourse import bass, bacc, tile, mybir
    from contextlib import ExitStack

    f32 = mybir.dt.float32
    f16 = mybir.dt.float16

    nc = bacc.Bacc()
    # stationary operand (S*input_hat).T, host-shuffled. xt0 carries the d0
    # chunk alone so the first matmuls gate on the fewest DMA packets; the
    # remaining d chunks ride one combined DMA (tile deps are whole-tile)
    xts_e = [
        nc.declare_dram_parameter(f"xt{d}", [128, 2, 256], f16, isOutput=False)
        for d in range(DCH)
    ]
    wt_e = nc.declare_dram_parameter("wt", [D, CPAD], f16, isOutput=False)
    out_e = nc.declare_dram_parameter("out", [B, CPAD], f16, isOutput=True)

    with tile.TileContext(nc) as tc, ExitStack() as ctx:
        cpool = ctx.enter_context(tc.tile_pool(name="const", bufs=1))
        xpool = ctx.enter_context(tc.tile_pool(name="xin", bufs=1))
        wpool = ctx.enter_context(tc.tile_pool(name="wts", bufs=2))
        opool = ctx.enter_context(tc.tile_pool(name="outb", bufs=4))
        pm = ctx.enter_context(tc.tile_pool(name="pm", bufs=8, space="PSUM"))

        # warm-up constants (keep the PE busy while the first DMAs land);
        # memset on gpsimd, whose preamble retires earliest
        wstat = cpool.tile([128, 128], f16)
        nc.gpsimd.memset(wstat[:], 1.0)
        wmov = cpool.tile([128, 128], f16)
        nc.gpsimd.memset(wmov[:], 1.0)

        # stationary operand rides the fast sync queue (gpsimd/scalar DMA
        # queues are ~4x slower): xd0 first, then w_d0, then the rest
        in_d = [
            xpool.tile([128, 2, 256], f16, name=f"in_d{d}") for d in range(DCH)
        ]

        warm = pm.tile([128, 512], f32, tag="pm")
        for i in range(NWARM):
            nc.tensor.matmul(
                warm[:, 0:128], wstat[:], wmov[:], start=True, stop=True
            )

        odmas = [nc.scalar.dma_start, nc.gpsimd.dma_start,
                 nc.sync.dma_start]

        def stat_ap(d, bt):
            h, j = divmod(bt, 2)
            return in_d[d][:, h, j * 128:(j + 1) * 128]

        # the first 1792 classes run as four 448-wide cold supers whose
        # weight tiles use unique single-use names (uniform w{d} tag sizes
        # are required: mixed sizes under one tag tax every matmul with
        # partial-overlap WAR tracking). The first matmul gates on only
        # xd0 + one 0.115MB weight chunk, and the per-d stationary chunks
        # interleave so every DMA in the cascade is ~0.6us.
        for ci in range(4):
            coff = 448 * ci
            vts = []
            for d in range(DCH):
                v_t = xpool.tile([128, GN], f16, name=f"v{ci}_{d}")
                if ci == 0 and d == 0:
                    nc.sync.dma_start(in_d[0][:], xts_e[0][:, :, :])
                nc.sync.dma_start(
                    v_t[:], wt_e[d * 128:(d + 1) * 128, coff:coff + GN]
                )
                if ci == 0 and d < DCH - 1:
                    nc.sync.dma_start(in_d[d + 1][:], xts_e[d + 1][:, :, :])
                vts.append(v_t)
            pmsc = [
                pm.tile([128, 512], f32, tag="pm", name=f"pmc{ci}_{bt}")
                for bt in range(BT)
            ]
            for d in range(DCH):
                for bt in range(BT):
                    nc.tensor.matmul(
                        pmsc[bt][:, 0:GN],
                        stat_ap(d, bt),
                        vts[d][:],
                        start=(d == 0),
                        stop=(d == DCH - 1),
                    )
            for bt in range(BT):
                obc = opool.tile([128, GN], f16, name=f"obc{ci}_{bt}")
                eng = nc.vector.tensor_copy if bt % 2 == 0 else nc.scalar.copy
                eng(obc[:], pmsc[bt][:, 0:GN])
                odmas[bt % 2](
                    out_e[bt * 128:(bt + 1) * 128, coff:coff + GN],
                    obc[:],
                )

        for s in range(1, SUP):
            wts = []
            for d in range(DCH):
                wt_t = wpool.tile([128, SUPC], f16, tag=f"w{d}")
                wsrc = wt_e[d * 128:(d + 1) * 128, s * SUPC:(s + 1) * SUPC]
                nc.sync.dma_start(wt_t[:], wsrc)
                wts.append(wt_t)

            for bt in range(BT):
                tail = s >= SUP - 2 and bt >= 2
                final = s == SUP - 1 and bt == BT - 1
                orows = out_e[bt * 128:(bt + 1) * 128, s * SUPC:(s + 1) * SUPC]

                if final:
                    # the very last tile uses 8 finer PSUM groups of 224 in
                    # g-outer order: each group's accumulation finishes
                    # early, its copy pipelines alongside later matmuls, and
                    # the drain starts right after the last matmul
                    GF = GN // 2
                    pms8 = [
                        pm.tile([128, 512], f32, tag="pm", name=f"pmF_{g}")
                        for g in range(2 * GRP)
                    ]
                    for g in range(2 * GRP):
                        for d in range(DCH):
                            nc.tensor.matmul(
                                pms8[g][:, 0:GF],
                                stat_ap(d, bt),
                                wts[d][:, g * GF:(g + 1) * GF],
                                start=(d == 0),
                                stop=(d == DCH - 1),
                            )
                    ob = opool.tile([128, SUPC], f16, tag="ob")
                    for g in range(2 * GRP):
                        eng = nc.vector.tensor_copy if g % 2 == 0 else nc.scalar.copy
                        eng(ob[:, g * GF:(g + 1) * GF], pms8[g][:, 0:GF])
                        if g == GRP - 1:
                            nc.sync.dma_start(
                                orows[:, 0:GRP * GF], ob[:, 0:GRP * GF]
                            )
                    nc.sync.dma_start(
                        orows[:, GRP * GF:2 * GRP * GF],
                        ob[:, GRP * GF:2 * GRP * GF],
                    )
                    continue

                pms = [
                    pm.tile([128, 512], f32, tag="pm", name=f"pm_{s}_{bt}_{g}")
                    for g in range(GRP)
                ]
                for d in range(DCH):
                    stat = stat_ap(d, bt)
                    for g in range(GRP):
                        nc.tensor.matmul(
                            pms[g][:, 0:GN],
                            stat,
                            wts[d][:, g * GN:(g + 1) * GN],
                            start=(d == 0),
                            stop=(d == DCH - 1),
                        )
                ob = opool.tile([128, SUPC], f16, tag="ob")
                for g in range(GRP):
                    eng = nc.vector.tensor_copy if g % 2 == 0 else nc.scalar.copy
                    eng(ob[:, g * GN:(g + 1) * GN], pms[g][:, 0:GN])
                if tail:
                    nc.sync.dma_start(orows, ob[:])
                elif s >= 3:
                    # late outputs avoid the gpsimd queue: its teardown
                    # DRAIN cost sits on the critical path
                    (nc.scalar.dma_start if (s * BT + bt) % 2 else nc.sync.dma_start)(
                        orows, ob[:]
                    )
                else:
                    odmas[(s * BT + bt) % 2](orows, ob[:])
    nc.finalize()
    return nc


def _get_nc():
    if "nc" not in _CACHE:
        _CACHE["nc"] = _build_nc()
    return _CACHE["nc"]


def kernel(input, label, weight):
    global LAST_RESULT
    from concourse.bass_utils import run_bass_kernel_spmd

    inp = np.asarray(input, dtype=np.float32)
    lbl = np.asarray(label).astype(np.int64)
    w = np.asarray(weight, dtype=np.float32)

    # host-side shard prep: normalize, transpose, fp16-cast
    xn = inp / np.maximum(np.linalg.norm(inp, axis=1, keepdims=True), 1e-12)
    xs = (S * xn).T.astype(np.float16).reshape(DCH, 128, 2, 256)
    xsd = [np.ascontiguousarray(xs[d]) for d in range(DCH)]  # [p, h, b%256]

    winv = 1.0 / np.maximum(np.linalg.norm(w, axis=1), 1e-12)
    wn = w * winv[:, None]
    wT = np.zeros((NCORES, D, CPAD), dtype=np.float16)
    wT[:, :, :CSH] = wn.reshape(NCORES, CSH, D).transpose(0, 2, 1)

    in_maps = [
        {**{f"xt{d}": xsd[d] for d in range(DCH)},
         "wt": np.ascontiguousarray(wT[k])}
        for k in range(NCORES)
    ]

    nc = _get_nc()
    res = run_bass_kernel_spmd(nc, in_maps, core_ids=list(range(NCORES)))
    LAST_RESULT = res
    outs = res.results

    full = np.empty((B, C), dtype=np.float32)
    for k in range(NCORES):
        blk = np.asarray(outs[k]["out"]).reshape(B, CPAD)[:, :CSH]
        full[:, k * CSH:(k + 1) * CSH] = blk.astype(np.float32)

    # apply the ArcFace margin to the 512 label positions (float64 on host)
    rows = np.arange(B)
    cosl = np.clip(full[rows, lbl].astype(np.float64) / S, -1.0, 1.0)
    sine = np.sqrt(np.clip(1.0 - cosl * cosl, 1e-9, 1.0))
    phi = cosl * COS_M - sine * SIN_M
    phi = np.where(cosl > TH, phi, cosl - MM)
    full[rows, lbl] = (S * phi).astype(np.float32)
    return full


# revision 80
# speedup vs baseline: 1.0404x; 1.0404x over previous
"""ArcFace loss kernel for 8 Trainium2 NeuronCores.

Strategy (class-parallel, Partial-FC style):
  - weight [100000, 512] is sharded along the class axis: 12500 classes per
    core (padded to 12544 = 7*1792). Shards are passed host-normalized,
    host-transposed ([D, Cpad]) in fp16 so the device streams them straight
    into the TensorEngine as the moving operand.
  - input [512, 512] is normalized and scaled by S on the host, transposed to
    [D, B] fp16, and broadcast to all cores as the stationary operand.
  - Each core computes out[b, c] = <S*in_hat_b, w_hat_c> for its class range
    in natural [B, Cpad] layout: per (super, b-tile) the 4 contraction chunks
    are accumulated into a 4-bank PSUM tile (4 groups of 448 classes), then
    evacuated to fp16 SBUF (VectorE + ScalarE split) and DMA'd out.
  - Stationary = input keeps LDWEIGHTS cheap; dummy warm-up matmuls hold
    the PE p-state (2.4GHz needs 3us of continuous work) while the first
    weight DMAs land on the sync queue, the only fast DMA queue. The final
    tile uses 8 finer PSUM groups, g-outer, so its copies pipeline with the
    matmuls and the output drain starts right after the last matmul.
  - The ArcFace margin only affects one element per row (b, label[b]); the
    host applies the phi transform to those 512 gathered cosines in float64.
"""

import math
import os
import sys

import numpy as np

for _p in ("/opt/trn_rl_repo",):
    if os.path.isdir(_p) and _p not in sys.path:
        sys.path.insert(0, _p)

S = 30.0
MARGIN = 0.5
COS_M = math.cos(MARGIN)
SIN_M = math.sin(MARGIN)
TH = math.cos(math.pi - MARGIN)
MM = math.sin(math.pi - MARGIN) * MARGIN

B, D, C = 512, 512, 100000
NCORES = 8
CSH = C // NCORES            # 12500 classes per core
SUP = 7                      # weight "supers" per core
SUPC = 1792                  # classes per super
CPAD = SUP * SUPC            # 12544
GRP = 4                      # PSUM banks (class groups) per super
GN = SUPC // GRP             # 448 classes per group
BT = B // 128                # 4 batch tiles
DCH = D // 128               # 4 contraction chunks
NWARM = 28                   # PE warm-up matmuls

LAST_RESULT = None
_CACHE = {}


def _build_nc():
    from concourse import bass, bacc, tile, mybir
    from contextlib import ExitStack

    f32 = mybir.dt.float32
    f16 = mybir.dt.float16

    nc = bacc.Bacc()
    # stationary operand (S*input_hat).T, host-shuffled. xt0 carries the d0
    # chunk alone so the first matmuls gate on the fewest DMA packets; the
    # remaining d chunks ride one combined DMA (tile deps are whole-tile)
    xt0_e = nc.declare_dram_parameter("xt0", [128, 2, 256], f16, isOutput=False)
    xt1_e = nc.declare_dram_parameter("xt1", [128, 2, 256], f16, isOutput=False)
    xt23_e = nc.declare_dram_parameter(
        "xt23", [128, 2, 2, 256], f16, isOutput=False
    )
    wt_e = nc.declare_dram_parameter("wt", [D, CPAD], f16, isOutput=False)
    out_e = nc.declare_dram_parameter("out", [B, CPAD], f16, isOutput=True)

    with tile.TileContext(nc) as tc, ExitStack() as ctx:
        cpool = ctx.enter_context(tc.tile_pool(name="const", bufs=1))
        xpool = ctx.enter_context(tc.tile_pool(name="xin", bufs=1))
        wpool = ctx.enter_context(tc.tile_pool(name="wts", bufs=2))
        opool = ctx.enter_context(tc.tile_pool(name="outb", bufs=4))
        pm = ctx.enter_context(tc.tile_pool(name="pm", bufs=8, space="PSUM"))

        # warm-up constants (keep the PE busy while the first DMAs land);
        # memset on gpsimd, whose preamble retires earliest
        wstat = cpool.tile([128, 128], f16)
        nc.gpsimd.memset(wstat[:], 1.0)
        wmov = cpool.tile([128, 128], f16)
        nc.gpsimd.memset(wmov[:], 1.0)

        # stationary operand rides the fast sync queue (gpsimd/scalar DMA
        # queues are ~4x slower): xd0 first, then w_d0, then the rest
        in_d0 = xpool.tile([128, 2, 256], f16)
        in_d1 = xpool.tile([128, 2, 256], f16)
        in_d23 = xpool.tile([128, 2, 2, 256], f16)

        warm = pm.tile([128, 512], f32, tag="pm")
        for i in range(NWARM):
            nc.tensor.matmul(
                warm[:, 0:128], wstat[:], wmov[:], start=True, stop=True
            )

        odmas = [nc.scalar.dma_start, nc.gpsimd.dma_start,
                 nc.sync.dma_start]

        def stat_ap(d, bt):
            h, j = divmod(bt, 2)
            if d == 0:
                return in_d0[:, h, j * 128:(j + 1) * 128]
            if d == 1:
                return in_d1[:, h, j * 128:(j + 1) * 128]
            return in_d23[:, h, d - 2, j * 128:(j + 1) * 128]

        # the first 1792 classes run as two 896-wide cold supers whose
        # weight tiles use unique single-use tags (uniform w{d} tag sizes
        # are required: mixed sizes under one tag tax every matmul with
        # partial-overlap WAR tracking). First matmul gates on only
        # xd0 + one 0.23MB weight chunk; the stationary chunks interleave
        # per-d so each d stage's deps land just in time.
        for ci, coff in enumerate((0, 896)):
            vts = []
            for d in range(DCH):
                v_t = xpool.tile([128, 896], f16, name=f"v{ci}_{d}")
                if ci == 0 and d == 0:
                    nc.sync.dma_start(in_d0[:], xt0_e[:, :, :])
                nc.sync.dma_start(
                    v_t[:], wt_e[d * 128:(d + 1) * 128, coff:coff + 896]
                )
                if ci == 0 and d == 0:
                    nc.sync.dma_start(in_d1[:], xt1_e[:, :, :])
                if ci == 0 and d == 1:
                    nc.sync.dma_start(in_d23[:], xt23_e[:, :, :, :])
                vts.append(v_t)
            pms2 = {}
            for bt in range(BT):
                for g in range(2):
                    pms2[(bt, g)] = pm.tile(
                        [128, 512], f32, tag="pm", name=f"pmc{ci}_{bt}_{g}"
                    )
            for d in range(DCH):
                for g in range(2):
                    for bt in range(BT):
                        nc.tensor.matmul(
                            pms2[(bt, g)][:, 0:GN],
                            stat_ap(d, bt),
                            vts[d][:, g * GN:(g + 1) * GN],
                            start=(d == 0),
                            stop=(d == DCH - 1),
                        )
            for bt in range(BT):
                obc = opool.tile([128, 896], f16, name=f"obc{ci}_{bt}")
                for g in range(2):
                    eng = nc.vector.tensor_copy if g % 2 == 0 else nc.scalar.copy
                    eng(obc[:, g * GN:(g + 1) * GN], pms2[(bt, g)][:, 0:GN])
                odmas[bt % 2](
                    out_e[bt * 128:(bt + 1) * 128, coff:coff + 896],
                    obc[:],
                )

        for s in range(1, SUP):
            wts = []
            for d in range(DCH):
                wt_t = wpool.tile([128, SUPC], f16, tag=f"w{d}")
                wsrc = wt_e[d * 128:(d + 1) * 128, s * SUPC:(s + 1) * SUPC]
                nc.sync.dma_start(wt_t[:], wsrc)
                wts.append(wt_t)

            for bt in range(BT):
                tail = s >= SUP - 2 and bt >= 2
                final = s == SUP - 1 and bt == BT - 1
                orows = out_e[bt * 128:(bt + 1) * 128, s * SUPC:(s + 1) * SUPC]

                if final:
                    # the very last tile uses 8 finer PSUM groups of 224 in
                    # g-outer order: each group's accumulation finishes
                    # early, its copy pipelines alongside later matmuls, and
                    # the drain starts right after the last matmul
                    GF = GN // 2
                    pms8 = [
                        pm.tile([128, 512], f32, tag="pm", name=f"pmF_{g}")
                        for g in range(2 * GRP)
                    ]
                    for g in range(2 * GRP):
                        for d in range(DCH):
                            nc.tensor.matmul(
                                pms8[g][:, 0:GF],
                                stat_ap(d, bt),
                                wts[d][:, g * GF:(g + 1) * GF],
                                start=(d == 0),
                                stop=(d == DCH - 1),
                            )
                    ob = opool.tile([128, SUPC], f16, tag="ob")
                    for g in range(2 * GRP):
                        eng = nc.vector.tensor_copy if g % 2 == 0 else nc.scalar.copy
                        eng(ob[:, g * GF:(g + 1) * GF], pms8[g][:, 0:GF])
                        if g == GRP - 1:
                            nc.sync.dma_start(
                                orows[:, 0:GRP * GF], ob[:, 0:GRP * GF]
                            )
                    nc.sync.dma_start(
                        orows[:, GRP * GF:2 * GRP * GF],
                        ob[:, GRP * GF:2 * GRP * GF],
                    )
                    continue

                pms = [
                    pm.tile([128, 512], f32, tag="pm", name=f"pm_{s}_{bt}_{g}")
                    for g in range(GRP)
                ]
                for d in range(DCH):
                    stat = stat_ap(d, bt)
                    for g in range(GRP):
                        nc.tensor.matmul(
                            pms[g][:, 0:GN],
                            stat,
                            wts[d][:, g * GN:(g + 1) * GN],
                            start=(d == 0),
                            stop=(d == DCH - 1),
                        )
                ob = opool.tile([128, SUPC], f16, tag="ob")
                for g in range(GRP):
                    eng = nc.vector.tensor_copy if g % 2 == 0 else nc.scalar.copy
                    eng(ob[:, g * GN:(g + 1) * GN], pms[g][:, 0:GN])
                if tail:
                    nc.sync.dma_start(orows, ob[:])
                elif s >= 3:
                    # late outputs avoid the gpsimd queue: its teardown
                    # DRAIN cost sits on the critical path
                    (nc.scalar.dma_start if (s * BT + bt) % 2 else nc.sync.dma_start)(
                        orows, ob[:]
                    )
                else:
                    odmas[(s * BT + bt) % 2](orows, ob[:])
    nc.finalize()
    return nc


def _get_nc():
    if "nc" not in _CACHE:
        _CACHE["nc"] = _build_nc()
    return _CACHE["nc"]


def kernel(input, label, weight):
    global LAST_RESULT
    from concourse.bass_utils import run_bass_kernel_spmd

    inp = np.asarray(input, dtype=np.float32)
    lbl = np.asarray(label).astype(np.int64)
    w = np.asarray(weight, dtype=np.float32)

    # host-side shard prep: normalize, transpose, fp16-cast
    xn = inp / np.maximum(np.linalg.norm(inp, axis=1, keepdims=True), 1e-12)
    xs = (S * xn).T.astype(np.float16).reshape(DCH, 128, 2, 256)
    xs0 = np.ascontiguousarray(xs[0])            # [p, h, b%256]
    xs1 = np.ascontiguousarray(xs[1])
    xs23 = np.ascontiguousarray(xs[2:].transpose(1, 2, 0, 3))  # [p, h, d-2, b%256]

    winv = 1.0 / np.maximum(np.linalg.norm(w, axis=1), 1e-12)
    wn = w * winv[:, None]
    wT = np.zeros((NCORES, D, CPAD), dtype=np.float16)
    wT[:, :, :CSH] = wn.reshape(NCORES, CSH, D).transpose(0, 2, 1)

    in_maps = [
        {"xt0": xs0, "xt1": xs1, "xt23": xs23, "wt": np.ascontiguousarray(wT[k])}
        for k in range(NCORES)
    ]

    nc = _get_nc()
    res = run_bass_kernel_spmd(nc, in_maps, core_ids=list(range(NCORES)))
    LAST_RESULT = res
    outs = res.results

    full = np.empty((B, C), dtype=np.float32)
    for k in range(NCORES):
        blk = np.asarray(outs[k]["out"]).reshape(B, CPAD)[:, :CSH]
        full[:, k * CSH:(k + 1) * CSH] = blk.astype(np.float32)

    # apply the ArcFace margin to the 512 label positions (float64 on host)
    rows = np.arange(B)
    cosl = np.clip(full[rows, lbl].astype(np.float64) / S, -1.0, 1.0)
    sine = np.sqrt(np.clip(1.0 - cosl * cosl, 1e-9, 1.0))
    phi = cosl * COS_M - sine * SIN_M
    phi = np.where(cosl > TH, phi, cosl - MM)
    full[rows, lbl] = (S * phi).astype(np.float32)
    return full


# revision 81
# speedup vs baseline: 1.0439x; 1.0033x over previous
"""ArcFace loss kernel for 8 Trainium2 NeuronCores.

Strategy (class-parallel, Partial-FC style):
  - weight [100000, 512] is sharded along the class axis: 12500 classes per
    core (padded to 12544 = 7*1792). Shards are passed host-normalized,
    host-transposed ([D, Cpad]) in fp16 so the device streams them straight
    into the TensorEngine as the moving operand.
  - input [512, 512] is normalized and scaled by S on the host, transposed to
    [D, B] fp16, and broadcast to all cores as the stationary operand.
  - Each core computes out[b, c] = <S*in_hat_b, w_hat_c> for its class range
    in natural [B, Cpad] layout: per (super, b-tile) the 4 contraction chunks
    are accumulated into a 4-bank PSUM tile (4 groups of 448 classes), then
    evacuated to fp16 SBUF (VectorE + ScalarE split) and DMA'd out.
  - Stationary = input keeps LDWEIGHTS cheap; dummy warm-up matmuls hold
    the PE p-state (2.4GHz needs 3us of continuous work) while the first
    weight DMAs land on the sync queue, the only fast DMA queue. The final
    tile uses 8 finer PSUM groups, g-outer, so its copies pipeline with the
    matmuls and the output drain starts right after the last matmul.
  - The ArcFace margin only affects one element per row (b, label[b]); the
    host applies the phi transform to those 512 gathered cosines in float64.
"""

import math
import os
import sys

import numpy as np

for _p in ("/opt/trn_rl_repo",):
    if os.path.isdir(_p) and _p not in sys.path:
        sys.path.insert(0, _p)

S = 30.0
MARGIN = 0.5
COS_M = math.cos(MARGIN)
SIN_M = math.sin(MARGIN)
TH = math.cos(math.pi - MARGIN)
MM = math.sin(math.pi - MARGIN) * MARGIN

B, D, C = 512, 512, 100000
NCORES = 8
CSH = C // NCORES            # 12500 classes per core
SUP = 7                      # weight "supers" per core
SUPC = 1792                  # classes per super
CPAD = SUP * SUPC            # 12544
GRP = 4                      # PSUM banks (class groups) per super
GN = SUPC // GRP             # 448 classes per group
BT = B // 128                # 4 batch tiles
DCH = D // 128               # 4 contraction chunks
NWARM = 32                   # PE warm-up matmuls

LAST_RESULT = None
_CACHE = {}


def _build_nc():
    from concourse import bass, bacc, tile, mybir
    from contextlib import ExitStack

    f32 = mybir.dt.float32
    f16 = mybir.dt.float16

    nc = bacc.Bacc()
    # stationary operand (S*input_hat).T, host-shuffled. xt0 carries the d0
    # chunk alone so the first matmuls gate on the fewest DMA packets; the
    # remaining d chunks ride one combined DMA (tile deps are whole-tile)
    xt0_e = nc.declare_dram_parameter("xt0", [128, 2, 256], f16, isOutput=False)
    xt1_e = nc.declare_dram_parameter("xt1", [128, 2, 256], f16, isOutput=False)
    xt23_e = nc.declare_dram_parameter(
        "xt23", [128, 2, 2, 256], f16, isOutput=False
    )
    wt_e = nc.declare_dram_parameter("wt", [D, CPAD], f16, isOutput=False)
    out_e = nc.declare_dram_parameter("out", [B, CPAD], f16, isOutput=True)

    with tile.TileContext(nc) as tc, ExitStack() as ctx:
        cpool = ctx.enter_context(tc.tile_pool(name="const", bufs=1))
        xpool = ctx.enter_context(tc.tile_pool(name="xin", bufs=1))
        wpool = ctx.enter_context(tc.tile_pool(name="wts", bufs=2))
        opool = ctx.enter_context(tc.tile_pool(name="outb", bufs=4))
        pm = ctx.enter_context(tc.tile_pool(name="pm", bufs=8, space="PSUM"))

        # warm-up constants (keep the PE busy while the first DMAs land);
        # memset on gpsimd, whose preamble retires earliest
        wstat = cpool.tile([128, 128], f16)
        nc.gpsimd.memset(wstat[:], 1.0)
        wmov = cpool.tile([128, 128], f16)
        nc.gpsimd.memset(wmov[:], 1.0)

        # stationary operand rides the fast sync queue (gpsimd/scalar DMA
        # queues are ~4x slower): xd0 first, then w_d0, then the rest
        in_d0 = xpool.tile([128, 2, 256], f16)
        in_d1 = xpool.tile([128, 2, 256], f16)
        in_d23 = xpool.tile([128, 2, 2, 256], f16)

        warm = pm.tile([128, 512], f32, tag="pm")
        for i in range(NWARM):
            nc.tensor.matmul(
                warm[:, 0:128], wstat[:], wmov[:], start=True, stop=True
            )

        odmas = [nc.scalar.dma_start, nc.gpsimd.dma_start,
                 nc.sync.dma_start]

        def stat_ap(d, bt):
            h, j = divmod(bt, 2)
            if d == 0:
                return in_d0[:, h, j * 128:(j + 1) * 128]
            if d == 1:
                return in_d1[:, h, j * 128:(j + 1) * 128]
            return in_d23[:, h, d - 2, j * 128:(j + 1) * 128]

        # the first 1792 classes run as two 896-wide cold supers whose
        # weight tiles use unique single-use tags (uniform w{d} tag sizes
        # are required: mixed sizes under one tag tax every matmul with
        # partial-overlap WAR tracking). First matmul gates on only
        # xd0 + one 0.23MB weight chunk; the stationary chunks interleave
        # per-d so each d stage's deps land just in time.
        for ci, coff in enumerate((0, 896)):
            vts = []
            for d in range(DCH):
                v_t = xpool.tile([128, 896], f16, name=f"v{ci}_{d}")
                if ci == 0 and d == 0:
                    nc.sync.dma_start(in_d0[:], xt0_e[:, :, :])
                nc.sync.dma_start(
                    v_t[:], wt_e[d * 128:(d + 1) * 128, coff:coff + 896]
                )
                if ci == 0 and d == 0:
                    nc.sync.dma_start(in_d1[:], xt1_e[:, :, :])
                if ci == 0 and d == 1:
                    nc.sync.dma_start(in_d23[:], xt23_e[:, :, :, :])
                vts.append(v_t)
            pms2 = {}
            for bt in range(BT):
                for g in range(2):
                    pms2[(bt, g)] = pm.tile(
                        [128, 512], f32, tag="pm", name=f"pmc{ci}_{bt}_{g}"
                    )
            for d in range(DCH):
                for g in range(2):
                    for bt in range(BT):
                        nc.tensor.matmul(
                            pms2[(bt, g)][:, 0:GN],
                            stat_ap(d, bt),
                            vts[d][:, g * GN:(g + 1) * GN],
                            start=(d == 0),
                            stop=(d == DCH - 1),
                        )
            for bt in range(BT):
                obc = opool.tile([128, 896], f16, name=f"obc{ci}_{bt}")
                for g in range(2):
                    eng = nc.vector.tensor_copy if g % 2 == 0 else nc.scalar.copy
                    eng(obc[:, g * GN:(g + 1) * GN], pms2[(bt, g)][:, 0:GN])
                nc.scalar.dma_start(
                    out_e[bt * 128:(bt + 1) * 128, coff:coff + 896],
                    obc[:],
                )

        for s in range(1, SUP):
            wts = []
            for d in range(DCH):
                wt_t = wpool.tile([128, SUPC], f16, tag=f"w{d}")
                wsrc = wt_e[d * 128:(d + 1) * 128, s * SUPC:(s + 1) * SUPC]
                nc.sync.dma_start(wt_t[:], wsrc)
                wts.append(wt_t)

            for bt in range(BT):
                tail = s >= SUP - 2 and bt >= 2
                final = s == SUP - 1 and bt == BT - 1
                orows = out_e[bt * 128:(bt + 1) * 128, s * SUPC:(s + 1) * SUPC]

                if final:
                    # the very last tile uses 8 finer PSUM groups of 224 in
                    # g-outer order: each group's accumulation finishes
                    # early, its copy pipelines alongside later matmuls, and
                    # the drain starts right after the last matmul
                    GF = GN // 2
                    pms8 = [
                        pm.tile([128, 512], f32, tag="pm", name=f"pmF_{g}")
                        for g in range(2 * GRP)
                    ]
                    for g in range(2 * GRP):
                        for d in range(DCH):
                            nc.tensor.matmul(
                                pms8[g][:, 0:GF],
                                stat_ap(d, bt),
                                wts[d][:, g * GF:(g + 1) * GF],
                                start=(d == 0),
                                stop=(d == DCH - 1),
                            )
                    ob = opool.tile([128, SUPC], f16, tag="ob")
                    for g in range(2 * GRP):
                        eng = nc.vector.tensor_copy if g % 2 == 0 else nc.scalar.copy
                        eng(ob[:, g * GF:(g + 1) * GF], pms8[g][:, 0:GF])
                        if g == GRP - 1:
                            nc.sync.dma_start(
                                orows[:, 0:GRP * GF], ob[:, 0:GRP * GF]
                            )
                    nc.sync.dma_start(
                        orows[:, GRP * GF:2 * GRP * GF],
                        ob[:, GRP * GF:2 * GRP * GF],
                    )
                    continue

                pms = [
                    pm.tile([128, 512], f32, tag="pm", name=f"pm_{s}_{bt}_{g}")
                    for g in range(GRP)
                ]
                for d in range(DCH):
                    stat = stat_ap(d, bt)
                    for g in range(GRP):
                        nc.tensor.matmul(
                            pms[g][:, 0:GN],
                            stat,
                            wts[d][:, g * GN:(g + 1) * GN],
                            start=(d == 0),
                            stop=(d == DCH - 1),
                        )
                ob = opool.tile([128, SUPC], f16, tag="ob")
                for g in range(GRP):
                    eng = nc.vector.tensor_copy if g % 2 == 0 else nc.scalar.copy
                    eng(ob[:, g * GN:(g + 1) * GN], pms[g][:, 0:GN])
                if tail:
                    nc.sync.dma_start(orows, ob[:])
                elif s >= 3:
                    # late outputs avoid the gpsimd queue: its teardown
                    # DRAIN cost sits on the critical path
                    (nc.scalar.dma_start if (s * BT + bt) % 2 else nc.sync.dma_start)(
                        orows, ob[:]
                    )
                else:
                    (nc.scalar.dma_start if bt % 2 else nc.sync.dma_start)(
                        orows, ob[:]
                    )
    nc.finalize()
    return nc


def _get_nc():
    if "nc" not in _CACHE:
        _CACHE["nc"] = _build_nc()
    return _CACHE["nc"]


def kernel(input, label, weight):
    global LAST_RESULT
    from concourse.bass_utils import run_bass_kernel_spmd

    inp = np.asarray(input, dtype=np.float32)
    lbl = np.asarray(label).astype(np.int64)
    w = np.asarray(weight, dtype=np.float32)

    # host-side shard prep: normalize, transpose, fp16-cast
    xn = inp / np.maximum(np.linalg.norm(inp, axis=1, keepdims=True), 1e-12)
    xs = (S * xn).T.astype(np.float16).reshape(DCH, 128, 2, 256)
    xs0 = np.ascontiguousarray(xs[0])            # [p, h, b%256]
    xs1 = np.ascontiguousarray(xs[1])
    xs23 = np.ascontiguousarray(xs[2:].transpose(1, 2, 0, 3))  # [p, h, d-2, b%256]

    winv = 1.0 / np.maximum(np.linalg.norm(w, axis=1), 1e-12)
    wn = w * winv[:, None]
    wT = np.zeros((NCORES, D, CPAD), dtype=np.float16)
    wT[:, :, :CSH] = wn.reshape(NCORES, CSH, D).transpose(0, 2, 1)

    in_maps = [
        {"xt0": xs0, "xt1": xs1, "xt23": xs23, "wt": np.ascontiguousarray(wT[k])}
        for k in range(NCORES)
    ]

    nc = _get_nc()
    res = run_bass_kernel_spmd(nc, in_maps, core_ids=list(range(NCORES)))
    LAST_RESULT = res
    outs = res.results

    full = np.empty((B, C), dtype=np.float32)
    for k in range(NCORES):
        blk = np.asarray(outs[k]["out"]).reshape(B, CPAD)[:, :CSH]
        full[:, k * CSH:(k + 1) * CSH] = blk.astype(np.float32)

    # apply the ArcFace margin to the 512 label positions (float64 on host)
    rows = np.arange(B)
    cosl = np.clip(full[rows, lbl].astype(np.float64) / S, -1.0, 1.0)
    sine = np.sqrt(np.clip(1.0 - cosl * cosl, 1e-9, 1.0))
    phi = cosl * COS_M - sine * SIN_M
    phi = np.where(cosl > TH, phi, cosl - MM)
    full[rows, lbl] = (S * phi).astype(np.float32)
    return full


# revision 82
# speedup vs baseline: 1.0574x; 1.0130x over previous
"""ArcFace loss kernel for 8 Trainium2 NeuronCores.

Strategy (class-parallel, Partial-FC style):
  - weight [100000, 512] is sharded along the class axis: 12500 classes per
    core (padded to 12544 = 7*1792). Shards are passed host-normalized,
    host-transposed ([D, Cpad]) in fp16 so the device streams them straight
    into the TensorEngine as the moving operand.
  - input [512, 512] is normalized and scaled by S on the host, transposed to
    [D, B] fp16, and broadcast to all cores as the stationary operand.
  - Each core computes out[b, c] = <S*in_hat_b, w_hat_c> for its class range
    in natural [B, Cpad] layout: per (super, b-tile) the 4 contraction chunks
    are accumulated into a 4-bank PSUM tile (4 groups of 448 classes), then
    evacuated to fp16 SBUF (VectorE + ScalarE split) and DMA'd out.
  - Stationary = input keeps LDWEIGHTS cheap; dummy warm-up matmuls hold
    the PE p-state (2.4GHz needs 3us of continuous work) while the first
    weight DMAs land on the sync queue, the only fast DMA queue. The final
    tile uses 8 finer PSUM groups, g-outer, so its copies pipeline with the
    matmuls and the output drain starts right after the last matmul.
  - The ArcFace margin only affects one element per row (b, label[b]); the
    host applies the phi transform to those 512 gathered cosines in float64.
"""

import math
import os
import sys

import numpy as np

for _p in ("/opt/trn_rl_repo",):
    if os.path.isdir(_p) and _p not in sys.path:
        sys.path.insert(0, _p)

S = 30.0
MARGIN = 0.5
COS_M = math.cos(MARGIN)
SIN_M = math.sin(MARGIN)
TH = math.cos(math.pi - MARGIN)
MM = math.sin(math.pi - MARGIN) * MARGIN

B, D, C = 512, 512, 100000
NCORES = 8
CSH = C // NCORES            # 12500 classes per core
SUP = 7                      # weight "supers" per core
SUPC = 1792                  # classes per super
CPAD = SUP * SUPC            # 12544
GRP = 4                      # PSUM banks (class groups) per super
GN = SUPC // GRP             # 448 classes per group
BT = B // 128                # 4 batch tiles
DCH = D // 128               # 4 contraction chunks
NWARM = 32                   # PE warm-up matmuls

LAST_RESULT = None
_CACHE = {}


def _build_nc():
    from concourse import bass, bacc, tile, mybir
    from contextlib import ExitStack

    f32 = mybir.dt.float32
    f16 = mybir.dt.float16

    nc = bacc.Bacc()
    # stationary operand (S*input_hat).T, host-shuffled. xt0 carries the d0
    # chunk alone so the first matmuls gate on the fewest DMA packets; the
    # remaining d chunks ride one combined DMA (tile deps are whole-tile)
    xt0_e = nc.declare_dram_parameter("xt0", [128, 2, 256], f16, isOutput=False)
    xt1_e = nc.declare_dram_parameter("xt1", [128, 2, 256], f16, isOutput=False)
    xt23_e = nc.declare_dram_parameter(
        "xt23", [128, 2, 2, 256], f16, isOutput=False
    )
    wt_e = nc.declare_dram_parameter("wt", [D, CPAD], f16, isOutput=False)
    out_e = nc.declare_dram_parameter("out", [B, CPAD], f16, isOutput=True)

    with tile.TileContext(nc) as tc, ExitStack() as ctx:
        cpool = ctx.enter_context(tc.tile_pool(name="const", bufs=1))
        xpool = ctx.enter_context(tc.tile_pool(name="xin", bufs=1))
        wpool = ctx.enter_context(tc.tile_pool(name="wts", bufs=2))
        opool = ctx.enter_context(tc.tile_pool(name="outb", bufs=4))
        pm = ctx.enter_context(tc.tile_pool(name="pm", bufs=8, space="PSUM"))

        # warm-up constants (keep the PE busy while the first DMAs land);
        # memset on gpsimd, whose preamble retires earliest
        wstat = cpool.tile([128, 128], f16)
        nc.gpsimd.memset(wstat[:], 1.0)
        wmov = cpool.tile([128, 128], f16)
        nc.gpsimd.memset(wmov[:], 1.0)

        # stationary operand rides the fast sync queue (gpsimd/scalar DMA
        # queues are ~4x slower): xd0 first, then w_d0, then the rest
        in_d0 = xpool.tile([128, 2, 256], f16)
        in_d1 = xpool.tile([128, 2, 256], f16)
        in_d23 = xpool.tile([128, 2, 2, 256], f16)

        warm = pm.tile([128, 512], f32, tag="pm")
        for i in range(NWARM):
            nc.tensor.matmul(
                warm[:, 0:128], wstat[:], wmov[:], start=True, stop=True
            )

        odmas = [nc.scalar.dma_start, nc.gpsimd.dma_start,
                 nc.sync.dma_start]

        def stat_ap(d, bt):
            h, j = divmod(bt, 2)
            if d == 0:
                return in_d0[:, h, j * 128:(j + 1) * 128]
            if d == 1:
                return in_d1[:, h, j * 128:(j + 1) * 128]
            return in_d23[:, h, d - 2, j * 128:(j + 1) * 128]

        # the first 1792 classes run as two 896-wide cold supers whose
        # weight tiles use unique single-use tags (uniform w{d} tag sizes
        # are required: mixed sizes under one tag tax every matmul with
        # partial-overlap WAR tracking). First matmul gates on only
        # xd0 + one 0.23MB weight chunk; the stationary chunks interleave
        # per-d so each d stage's deps land just in time.
        for ci, coff in enumerate((0, 896)):
            vts = []
            for d in range(DCH):
                v_t = xpool.tile([128, 896], f16, name=f"v{ci}_{d}")
                if ci == 0 and d == 0:
                    nc.sync.dma_start(in_d0[:], xt0_e[:, :, :])
                nc.sync.dma_start(
                    v_t[:], wt_e[d * 128:(d + 1) * 128, coff:coff + 896]
                )
                if ci == 0 and d == 0:
                    nc.sync.dma_start(in_d1[:], xt1_e[:, :, :])
                if ci == 0 and d == 1:
                    nc.sync.dma_start(in_d23[:], xt23_e[:, :, :, :])
                vts.append(v_t)
            pms2 = {}
            for bt in range(BT):
                for g in range(2):
                    pms2[(bt, g)] = pm.tile(
                        [128, 512], f32, tag="pm", name=f"pmc{ci}_{bt}_{g}"
                    )
            for d in range(DCH):
                for g in range(2):
                    for bt in range(BT):
                        nc.tensor.matmul(
                            pms2[(bt, g)][:, 0:GN],
                            stat_ap(d, bt),
                            vts[d][:, g * GN:(g + 1) * GN],
                            start=(d == 0),
                            stop=(d == DCH - 1),
                        )
            for bt in range(BT):
                obc = opool.tile([128, 896], f16, name=f"obc{ci}_{bt}")
                for g in range(2):
                    eng = nc.vector.tensor_copy if g % 2 == 0 else nc.scalar.copy
                    eng(obc[:, g * GN:(g + 1) * GN], pms2[(bt, g)][:, 0:GN])
                nc.scalar.dma_start(
                    out_e[bt * 128:(bt + 1) * 128, coff:coff + 896],
                    obc[:],
                )

        for s in range(1, SUP):
            wts = []
            for d in range(DCH):
                wt_t = wpool.tile([128, SUPC], f16, tag=f"w{d}")
                wsrc = wt_e[d * 128:(d + 1) * 128, s * SUPC:(s + 1) * SUPC]
                nc.sync.dma_start(wt_t[:], wsrc)
                wts.append(wt_t)

            for bt in range(BT):
                tail = s >= SUP - 2 and bt >= 2
                final = s == SUP - 1 and bt == BT - 1
                orows = out_e[bt * 128:(bt + 1) * 128, s * SUPC:(s + 1) * SUPC]

                if final:
                    # the very last tile uses 8 finer PSUM groups of 224 in
                    # g-outer order: each group's accumulation finishes
                    # early, its copy pipelines alongside later matmuls, and
                    # the drain starts right after the last matmul
                    GF = GN // 2
                    pms8 = [
                        pm.tile([128, 512], f32, tag="pm", name=f"pmF_{g}")
                        for g in range(2 * GRP)
                    ]
                    for g in range(2 * GRP):
                        for d in range(DCH):
                            nc.tensor.matmul(
                                pms8[g][:, 0:GF],
                                stat_ap(d, bt),
                                wts[d][:, g * GF:(g + 1) * GF],
                                start=(d == 0),
                                stop=(d == DCH - 1),
                            )
                    ob = opool.tile([128, SUPC], f16, tag="ob")
                    for g in range(2 * GRP):
                        eng = nc.vector.tensor_copy if g % 2 == 0 else nc.scalar.copy
                        eng(ob[:, g * GF:(g + 1) * GF], pms8[g][:, 0:GF])
                        if g == GRP - 1:
                            nc.sync.dma_start(
                                orows[:, 0:GRP * GF], ob[:, 0:GRP * GF]
                            )
                    nc.sync.dma_start(
                        orows[:, GRP * GF:2 * GRP * GF],
                        ob[:, GRP * GF:2 * GRP * GF],
                    )
                    continue

                pms = [
                    pm.tile([128, 512], f32, tag="pm", name=f"pm_{s}_{bt}_{g}")
                    for g in range(GRP)
                ]
                for d in range(DCH):
                    stat = stat_ap(d, bt)
                    for g in range(GRP):
                        nc.tensor.matmul(
                            pms[g][:, 0:GN],
                            stat,
                            wts[d][:, g * GN:(g + 1) * GN],
                            start=(d == 0),
                            stop=(d == DCH - 1),
                        )
                ob = opool.tile([128, SUPC], f16, tag="ob")
                for g in range(GRP):
                    eng = nc.vector.tensor_copy if g % 2 == 0 else nc.scalar.copy
                    eng(ob[:, g * GN:(g + 1) * GN], pms[g][:, 0:GN])
                if tail:
                    nc.sync.dma_start(orows, ob[:])
                elif s >= 3:
                    # late outputs avoid the gpsimd queue: its teardown
                    # DRAIN cost sits on the critical path
                    (nc.scalar.dma_start if (s * BT + bt) % 2 else nc.sync.dma_start)(
                        orows, ob[:]
                    )
                else:
                    # s1/s2 outs must NOT ride sync: the DMA issue blocks
                    # the sync engine on the copy semaphore, stalling the
                    # next super's weight issues while the queue is still
                    # catching up from the cold phase
                    (nc.gpsimd.dma_start if bt % 2 else nc.scalar.dma_start)(
                        orows, ob[:]
                    )
    nc.finalize()
    return nc


def _get_nc():
    if "nc" not in _CACHE:
        _CACHE["nc"] = _build_nc()
    return _CACHE["nc"]


def kernel(input, label, weight):
    global LAST_RESULT
    from concourse.bass_utils import run_bass_kernel_spmd

    inp = np.asarray(input, dtype=np.float32)
    lbl = np.asarray(label).astype(np.int64)
    w = np.asarray(weight, dtype=np.float32)

    # host-side shard prep: normalize, transpose, fp16-cast
    xn = inp / np.maximum(np.linalg.norm(inp, axis=1, keepdims=True), 1e-12)
    xs = (S * xn).T.astype(np.float16).reshape(DCH, 128, 2, 256)
    xs0 = np.ascontiguousarray(xs[0])            # [p, h, b%256]
    xs1 = np.ascontiguousarray(xs[1])
    xs23 = np.ascontiguousarray(xs[2:].transpose(1, 2, 0, 3))  # [p, h, d-2, b%256]

    winv = 1.0 / np.maximum(np.linalg.norm(w, axis=1), 1e-12)
    wn = w * winv[:, None]
    wT = np.zeros((NCORES, D, CPAD), dtype=np.float16)
    wT[:, :, :CSH] = wn.reshape(NCORES, CSH, D).transpose(0, 2, 1)

    in_maps = [
        {"xt0": xs0, "xt1": xs1, "xt23": xs23, "wt": np.ascontiguousarray(wT[k])}
        for k in range(NCORES)
    ]

    nc = _get_nc()
    res = run_bass_kernel_spmd(nc, in_maps, core_ids=list(range(NCORES)))
    LAST_RESULT = res
    outs = res.results

    full = np.empty((B, C), dtype=np.float32)
    for k in range(NCORES):
        blk = np.asarray(outs[k]["out"]).reshape(B, CPAD)[:, :CSH]
        full[:, k * CSH:(k + 1) * CSH] = blk.astype(np.float32)

    # apply the ArcFace margin to the 512 label positions (float64 on host)
    rows = np.arange(B)
    cosl = np.clip(full[rows, lbl].astype(np.float64) / S, -1.0, 1.0)
    sine = np.sqrt(np.clip(1.0 - cosl * cosl, 1e-9, 1.0))
    phi = cosl * COS_M - sine * SIN_M
    phi = np.where(cosl > TH, phi, cosl - MM)
    full[rows, lbl] = (S * phi).astype(np.float32)
    return full
